# revision 4
# baseline (speedup 1.0000x reference)
"""Trainium2 Bass kernel for CrossAttentionAudio2T.

Sharding: 8 cores = batch B=4  x  2 head-groups (6 heads each).
Per core: feature-major activations, f32r matmuls (tf32-like, fp32 accum) for
projections/scores, bf16 for the attn*V pass; softmax without max-subtraction
(scores are in [-3, 3]); denominator via a ones-column-augmented V;
normalization via reciprocal + PE outer-product broadcast.
"""
import sys
sys.path.insert(0, '/opt/trn_rl_repo')
import numpy as np
import concourse.bass as bass
import concourse.tile as tile
from concourse import bacc, mybir
from concourse.bass_utils import run_bass_kernel_spmd

F32 = mybir.dt.float32
F32R = mybir.dt.float32r
BF16 = mybir.dt.bfloat16
Exp = mybir.ActivationFunctionType.Exp

B, T, N, NA, ST, D, H = 4, 8, 196, 196, 8, 768, 12
HD = D // H                 # 64
SCALE = float(HD ** -0.5)
L = T * N                   # 1568 (= NA*ST)
GH = H // 2                 # 6 heads per core
GD = GH * HD                # 384
P = 128
QT = 392                    # q free-dim tile; 4 per L
NQT = 4
LKP = 1664                  # keys padded to 13*128
NKC = 13                    # key chunks of 128
TAILV = L - (NKC - 1) * P   # 32 valid keys in the last chunk
DC = D // P                 # 6 contraction chunks
MT = GD // P                # 3 q/k m-tiles
OMT = D // P                # 6 output m-tiles
VW = GH * (HD + 1)          # 390: V-augmented row width

_CACHE = {}


def _build():
    nc = bacc.Bacc("TRN2", target_bir_lowering=False, debug=False, num_devices=8)
    d = {}
    d['xT'] = nc.dram_tensor("xT", (D, L), F32R, kind="ExternalInput").ap()
    d['aT'] = nc.dram_tensor("aT", (D, L), F32R, kind="ExternalInput").ap()
    d['sq'] = nc.dram_tensor("sq", (D, N), F32R, kind="ExternalInput").ap()
    d['tq'] = nc.dram_tensor("tq", (D, T), F32R, kind="ExternalInput").ap()
    d['sk'] = nc.dram_tensor("sk", (D, NA), F32R, kind="ExternalInput").ap()
    d['tk'] = nc.dram_tensor("tk", (D, ST), F32R, kind="ExternalInput").ap()
    d['wq'] = nc.dram_tensor("wq", (D, GD), F32R, kind="ExternalInput").ap()
    d['wk'] = nc.dram_tensor("wk", (D, GD), F32R, kind="ExternalInput").ap()
    d['wv'] = nc.dram_tensor("wv", (D, GD), F32R, kind="ExternalInput").ap()
    d['wp'] = nc.dram_tensor("wp", (GD, D), F32R, kind="ExternalInput").ap()
    d['qb'] = nc.dram_tensor("qb", (GD, 1), F32, kind="ExternalInput").ap()
    d['kb'] = nc.dram_tensor("kb", (GD, 1), F32, kind="ExternalInput").ap()
    # cst: cols 0:64 = 1.0, cols 64:160 = 0.0 (f32 bits)
    d['cst'] = nc.dram_tensor("cst", (P, 160), F32R, kind="ExternalInput").ap()
    # cstb: cols 0:8 = 1.0, cols 8:400 = 0.0 (bf16)
    d['cstb'] = nc.dram_tensor("cstb", (P, 400), BF16, kind="ExternalInput").ap()
    d['outT'] = nc.dram_tensor("outT", (D, L), F32, kind="ExternalOutput").ap()

    with tile.TileContext(nc) as tc:
        _body(tc, d)
    nc.compile()
    return nc


def _body(tc, d):
    nc = tc.nc
    with tc.tile_pool(name="wpool", bufs=1) as wpool, \
         tc.tile_pool(name="qkvpool", bufs=1) as qkvp, \
         tc.tile_pool(name="pspool", bufs=1, space="PSUM") as psp:

        # ---------------- weight / const loads ----------------
        wq_t, wk_t, wv_t, wp_t = [], [], [], []
        for c in range(DC):
            t = wpool.tile([P, GD], F32R, name=f"wq{c}", tag=f"wq{c}")
            nc.sync.dma_start(t[:], d['wq'][c * P:(c + 1) * P, :])
            wq_t.append(t)
            t = wpool.tile([P, GD], F32R, name=f"wk{c}", tag=f"wk{c}")
            nc.sync.dma_start(t[:], d['wk'][c * P:(c + 1) * P, :])
            wk_t.append(t)
            t = wpool.tile([P, GD], F32R, name=f"wv{c}", tag=f"wv{c}")
            nc.sync.dma_start(t[:], d['wv'][c * P:(c + 1) * P, :])
            wv_t.append(t)
        for c in range(MT):
            t = wpool.tile([P, D], F32R, name=f"wp{c}", tag=f"wp{c}")
            nc.sync.dma_start(t[:], d['wp'][c * P:(c + 1) * P, :])
            wp_t.append(t)
        qb_t, kb_t = [], []
        for m in range(MT):
            t = wpool.tile([P, 1], F32, name=f"qb{m}", tag=f"qb{m}")
            nc.sync.dma_start(t[:], d['qb'][m * P:(m + 1) * P, :])
            qb_t.append(t)
            t = wpool.tile([P, 1], F32, name=f"kb{m}", tag=f"kb{m}")
            nc.sync.dma_start(t[:], d['kb'][m * P:(m + 1) * P, :])
            kb_t.append(t)
        ones_col = wpool.tile([1, HD], F32R, name="ones_col", tag="ones_col")
        nc.sync.dma_start(ones_col[:], d['cst'][0:1, 0:HD])

        # persistent q/k/o/v tiles
        qt, kt, ot, vt = [], [], [], []
        for m in range(MT):
            qt.append(qkvp.tile([P, L], F32R, name=f"qt{m}", tag=f"qt{m}"))
            kt.append(qkvp.tile([P, LKP], F32R, name=f"kt{m}", tag=f"kt{m}"))
            ot.append(qkvp.tile([P, L], F32R, name=f"ot{m}", tag=f"ot{m}"))
        for kc in range(NKC):
            vt.append(qkvp.tile([P, VW], BF16, name=f"vt{kc}", tag=f"vt{kc}"))

        with tc.tile_pool(name="xapool", bufs=1) as xap, \
             tc.tile_pool(name="pospool", bufs=2) as smp:
            # -------- activations + pos adds --------
            xp, ap = [], []
            for c in range(DC):
                cs = slice(c * P, (c + 1) * P)
                xt = xap.tile([P, L], F32R, name=f"xp{c}", tag=f"xp{c}")
                nc.sync.dma_start(xt[:], d['xT'][cs, :])
                sqc = smp.tile([P, N], F32R, name=f"sqc{c}", tag="sqc")
                nc.sync.dma_start(sqc[:], d['sq'][cs, :])
                tqc = smp.tile([P, T], F32R, name=f"tqc{c}", tag="tqc")
                nc.sync.dma_start(tqc[:], d['tq'][cs, :])
                xv = xt[:].rearrange("p (a b) -> p a b", a=T)        # token = t*N + n
                nc.vector.tensor_add(xv, xv, sqc[:].unsqueeze(1).broadcast_to([P, T, N]))
                nc.vector.tensor_add(xv, xv, tqc[:].unsqueeze(2).broadcast_to([P, T, N]))
                xp.append(xt)

                at = xap.tile([P, L], F32R, name=f"ap{c}", tag=f"ap{c}")
                nc.sync.dma_start(at[:], d['aT'][cs, :])
                skc = smp.tile([P, NA], F32R, name=f"skc{c}", tag="skc")
                nc.sync.dma_start(skc[:], d['sk'][cs, :])
                tkc = smp.tile([P, ST], F32R, name=f"tkc{c}", tag="tkc")
                nc.sync.dma_start(tkc[:], d['tk'][cs, :])
                av = at[:].rearrange("p (a b) -> p a b", a=NA)       # token = n*ST + t
                nc.vector.tensor_add(av, av, skc[:].unsqueeze(2).broadcast_to([P, NA, ST]))
                nc.vector.tensor_add(av, av, tkc[:].unsqueeze(1).broadcast_to([P, NA, ST]))
                ap.append(at)

            # -------- q/k/v projections --------
            for m in range(MT):
                ms = slice(m * P, (m + 1) * P)
                for qi in range(NQT):
                    qs = slice(qi * QT, (qi + 1) * QT)
                    pp = psp.tile([P, 2, 512], F32, name="qpp", tag="oo", bufs=2)
                    for c in range(DC):
                        nc.tensor.matmul(pp[:, 0, 0:QT], wq_t[c][:, ms], xp[c][:, qs],
                                         start=(c == 0), stop=(c == DC - 1))
                    nc.vector.tensor_scalar_add(qt[m][:, qs], pp[:, 0, 0:QT], qb_t[m][:])
            for m in range(MT):
                ms = slice(m * P, (m + 1) * P)
                for qi in range(NQT):
                    qs = slice(qi * QT, (qi + 1) * QT)
                    pp = psp.tile([P, 2, 512], F32, name="kpp", tag="oo", bufs=2)
                    for c in range(DC):
                        nc.tensor.matmul(pp[:, 0, 0:QT], wk_t[c][:, ms], ap[c][:, qs],
                                         start=(c == 0), stop=(c == DC - 1))
                    nc.vector.tensor_scalar_add(kt[m][:, qs], pp[:, 0, 0:QT], kb_t[m][:])
                nc.sync.dma_start(kt[m][:, L:LKP], d['cst'][:, 64:160])  # zero pad cols
            for kc in range(NKC):
                nrow = P if kc < NKC - 1 else TAILV
                ks = slice(kc * P, kc * P + nrow)
                pp = psp.tile([P, 2, 512], F32, name="vpp", tag="oo", bufs=2)
                for c in range(DC):
                    nc.tensor.matmul(pp[0:nrow, 0, 0:GD], ap[c][:, ks], wv_t[c][:],
                                     start=(c == 0), stop=(c == DC - 1))
                tv = vt[kc][:].rearrange("p (h c) -> p h c", h=GH)
                nc.vector.tensor_copy(tv[0:nrow, :, 0:HD],
                                      pp[0:nrow, 0, 0:GD].rearrange("p (h c) -> p h c", h=GH))
                nc.sync.dma_start(tv[0:nrow, :, HD:HD + 1], d['cstb'][0:nrow, 0:GH].unsqueeze(2))
                if nrow < P:  # zero padded V rows (incl. their ones column)
                    nc.sync.dma_start(vt[kc][nrow:P, :], d['cstb'][nrow:P, 8:8 + VW])

        # ---------------- attention ----------------
        with tc.tile_pool(name="espool", bufs=3) as esp, \
             tc.tile_pool(name="attnsmall", bufs=3) as asp, \
             tc.tile_pool(name="obpool", bufs=3) as obp:
            for h in range(GH):
                mt_, po = h // 2, HD * (h % 2)
                hs = slice(po, po + HD)
                vs = slice(h * (HD + 1), (h + 1) * (HD + 1))
                for half in range(2):
                    o_ps = psp.tile([P, 2, 512], F32, name="o_ps", tag="oo", bufs=2)
                    for kc in range(NKC):
                        ks = slice(kc * P, (kc + 1) * P)
                        s_ps = psp.tile([P, 2, 512], F32, name="s_ps", tag="ss", bufs=2)
                        for q2 in range(2):
                            qi = 2 * half + q2
                            qs = slice(qi * QT, (qi + 1) * QT)
                            nc.tensor.matmul(s_ps[:, q2, 0:QT], kt[mt_][hs, ks],
                                             qt[mt_][hs, qs], start=True, stop=True)
                        es = esp.tile([P, 2 * QT], BF16, name="es", tag="es")
                        nc.scalar.activation(es[:].rearrange("p (a b) -> p a b", a=2),
                                             s_ps[:, :, 0:QT], Exp, scale=SCALE)
                        for q2 in range(2):
                            nc.tensor.matmul(o_ps[0:HD + 1, q2, 0:QT], vt[kc][:, vs],
                                             es[:, q2 * QT:(q2 + 1) * QT],
                                             start=(kc == 0), stop=(kc == NKC - 1))
                    for q2 in range(2):
                        qi = 2 * half + q2
                        qs = slice(qi * QT, (qi + 1) * QT)
                        rcp = asp.tile([1, QT], F32R, name="rcp", tag="rcp")
                        with nc.allow_low_precision(reason="f32r softmax denom"):
                            nc.vector.reciprocal(rcp[:], o_ps[HD:HD + 1, q2, 0:QT])
                        rep_ps = psp.tile([HD, 2, 512], F32, name="rep_ps", tag="ss", bufs=2)
                        nc.tensor.matmul(rep_ps[:, 0, 0:QT], ones_col[:], rcp[:],
                                         start=True, stop=True)
                        rep_sb = asp.tile([HD, QT], F32, name="rep_sb", tag="rep_sb")
                        nc.vector.tensor_copy(rep_sb[:], rep_ps[:, 0, 0:QT])
                        nc.vector.tensor_mul(ot[mt_][hs, qs], o_ps[0:HD, q2, 0:QT], rep_sb[:])

            # ---------------- output projection ----------------
            for m in range(OMT):
                ms = slice(m * P, (m + 1) * P)
                for qi in range(NQT):
                    qs = slice(qi * QT, (qi + 1) * QT)
                    op = psp.tile([P, 2, 512], F32, name="op", tag="oo", bufs=2)
                    for c in range(MT):
                        nc.tensor.matmul(op[:, 0, 0:QT], wp_t[c][:, ms], ot[c][:, qs],
                                         start=(c == 0), stop=(c == MT - 1))
                    ob = obp.tile([P, QT], F32, name="ob", tag="ob")
                    nc.vector.tensor_copy(ob[:], op[:, 0, 0:QT])
                    nc.sync.dma_start(d['outT'][ms, qs], ob[:])


def _prep(inputs):
    """Host-side sharding prep: per-core input maps."""
    t_x = np.asarray(inputs['t_x'], np.float32)
    audio = np.asarray(inputs['audio'], np.float32)
    sq = np.ascontiguousarray(np.asarray(inputs['vmae_space_pos'], np.float32).T)
    tq = np.ascontiguousarray(np.asarray(inputs['vmae_temporal_pos'], np.float32).T)
    sk = np.ascontiguousarray(np.asarray(inputs['audio_space_pos'], np.float32).T)
    tk = np.ascontiguousarray(np.asarray(inputs['audio_temporal_pos'], np.float32).T)
    q_w = np.asarray(inputs['q_w'], np.float32)
    q_b = np.asarray(inputs['q_b'], np.float32)
    kv_w = np.asarray(inputs['kv_w'], np.float32)
    kv_b = np.asarray(inputs['kv_b'], np.float32)
    proj_w = np.asarray(inputs['proj_w'], np.float32)

    import ml_dtypes
    cst = np.zeros((P, 160), np.float32)
    cst[:, 0:64] = 1.0
    cstb = np.zeros((P, 400), ml_dtypes.bfloat16)
    cstb[:, 0:8] = 1.0

    xT = [np.ascontiguousarray(t_x[b].T) for b in range(B)]
    aT = [np.ascontiguousarray(
        audio[2:, b * ST:(b + 1) * ST, :].reshape(NA * ST, D).T) for b in range(B)]

    in_maps = []
    for c in range(8):
        b, g = c // 2, c % 2
        gs = slice(g * GD, (g + 1) * GD)
        in_maps.append({
            'xT': xT[b], 'aT': aT[b], 'sq': sq, 'tq': tq, 'sk': sk, 'tk': tk,
            'wq': np.ascontiguousarray(q_w[gs, :].T),
            'wk': np.ascontiguousarray(kv_w[0:D, :][gs, :].T),
            'wv': np.ascontiguousarray(kv_w[D:2 * D, :][gs, :].T),
            'wp': np.ascontiguousarray(proj_w[:, gs].T),
            'qb': np.ascontiguousarray(q_b[gs].reshape(GD, 1)),
            'kb': np.ascontiguousarray(kv_b[0:D][gs].reshape(GD, 1)),
            'cst': cst, 'cstb': cstb,
        })
    final_bias = (np.asarray(inputs['proj_b'], np.float32)
                  + proj_w @ kv_b[D:2 * D])
    return in_maps, final_bias


def kernel(**inputs):
    if 'nc' not in _CACHE:
        _CACHE['nc'] = _build()
    nc = _CACHE['nc']
    in_maps, final_bias = _prep(inputs)
    res = run_bass_kernel_spmd(nc, in_maps, core_ids=list(range(8)))
    out = np.empty((B, L, D), np.float32)
    for b in range(B):
        acc = res.results[2 * b]['outT'] + res.results[2 * b + 1]['outT']
        out[b] = acc.T + final_bias
    return out
